# revision 5
# baseline (speedup 1.0000x reference)
"""Pairwise KL divergence kernel for Trainium2, SPMD across 8 NeuronCores.

out[n, m] = sum_d a[n,d]*(log a[n,d] - log b[m,d])
          = ent[n] - (a @ log(b)^T)[n, m],  ent = rowsum(a * log a)

Sharding: a (and output rows) split 8 ways; b replicated.
Per core: a_shard (1024, 64), b (8192, 64) -> out_shard (1024, 8192).

Structure (chunk-pipelined so the 32 MB output DMA starts ~6 us in and
never starves — it is the long pole at ~93 us of ~360 GB/s per core):
  - a prologue: load a natural (128p, 8t, 64d); la = Ln(a) [ACT];
    prod = a*la [DVE]; ent[128,8] per-tile reduce [DVE]; aT (f32r) via
    8 PE transposes + ACT copies.
  - for each of 8 b-chunks (1024 rows, double-buffered load):
      transpose the RAW chunk on PE, then one ACT Ln op per 4-tile group
      evacuates PSUM -> lbT (float32r) — Ln fused into the transpose
      evacuation, no separate lb tile or copies.
      for each of 8 n-tiles: 2 fp32r matmuls -> psum (128, 2, 512);
      evac fused with the entropy term (ACT Identity w/ bias, alternating
      with DVE tensor_scalar) -> 512 KB DMA to HBM (4 KB/partition lines).
"""

import numpy as np

N, M, D = 8192, 8192, 64
NCORES = 8
NSHARD = N // NCORES          # 1024 rows of a per core
NT = NSHARD // 128            # 8 n-tiles per core
HCH = 1024                    # b rows per chunk
NCH = M // HCH                # 8 chunks
CT = HCH // 128               # 8 b row-tiles per chunk

# matmul operand dtype: "fp32" (safe, 4 cyc/row) or "fp32r" (1 cyc/row)
MM_DTYPE = "fp32r"

_CACHE = {}


def _build(mm_dtype):
    from contextlib import ExitStack

    import concourse.bacc as bacc_mod
    import concourse.bass as bass
    import concourse.mybir as mybir
    import concourse.tile as tile
    from concourse.masks import make_identity

    FP32 = mybir.dt.float32
    AF = mybir.ActivationFunctionType
    ALU = mybir.AluOpType
    AX = mybir.AxisListType

    nc = bacc_mod.Bacc()
    a_d = nc.dram_tensor("a", [NSHARD, D], FP32, kind="ExternalInput")
    b_d = nc.dram_tensor("b", [M, D], FP32, kind="ExternalInput")
    out_d = nc.dram_tensor("out", [NSHARD, M], FP32, kind="ExternalOutput")

    # fp32r matmul operands must be *produced* as float32r (the BIR verifier
    # requires the producing instruction to round) — the ACT evacuations of
    # the PE transposes write float32r directly.
    MMDT = mybir.dt.float32r if mm_dtype == "fp32r" else FP32

    with tile.TileContext(nc) as tc, ExitStack() as ctx:
        consts = ctx.enter_context(tc.tile_pool(name="consts", bufs=1))
        apool = ctx.enter_context(tc.tile_pool(name="apool", bufs=1))
        bpool = ctx.enter_context(tc.tile_pool(name="bpool", bufs=2))
        lbtp = ctx.enter_context(tc.tile_pool(name="lbtp", bufs=2))
        tpsum = ctx.enter_context(tc.tile_pool(name="tpsum", bufs=2, space="PSUM"))
        mmps = ctx.enter_context(tc.tile_pool(name="mmps", bufs=3, space="PSUM"))
        stage = ctx.enter_context(tc.tile_pool(name="stage", bufs=6))

        ident = consts.tile([128, 128], FP32)
        make_identity(nc, ident)
        # Dummy transpose so PE observes the gpsimd (ident) sem here: the
        # matmul/LDW struct only carries ONE sync wait, so later transposes
        # must each need at most one sem (codegen: "Too many sync waits").
        warm = tpsum.tile([128, 128], FP32, tag="tp")
        nc.tensor.transpose(warm, ident, ident)

        # Linear chunk loads: partition p holds b rows h*1024 + 8p .. 8p+7 as
        # one contiguous 2 KB line (vs 256 B lines for the natural layout,
        # which eat ~10 us of DMA-engine time at the <512 B RMW penalty).
        b_r = b_d[:, :].rearrange("(c p j) d -> p c (j d)", c=NCH, p=128)

        def load_chunk(h):
            t = bpool.tile([128, CT, D], FP32, tag="b_nat")
            nc.sync.dma_start(
                out=t, in_=b_r[:, h, :].rearrange("p (j d) -> p j d", d=D)
            )
            return t

        # ---------------- a prologue ----------------
        # a loads FIRST: PE's first queued work (aT transposes) depends on it,
        # and the slow natural-layout load (256 B lines) must not trail b0.
        a_nat = apool.tile([128, NT, D], FP32)        # row t*128+p at [p, t, :]
        nc.sync.dma_start(out=a_nat, in_=a_d[:, :].rearrange("(t p) d -> p t d", p=128))
        b_tiles = [load_chunk(0), None]
        la = apool.tile([128, NT, D], FP32)
        nc.scalar.activation(la, a_nat, AF.Ln)
        prod = apool.tile([128, NT, D], FP32)
        nc.vector.tensor_mul(prod, a_nat, la)
        ent = apool.tile([128, NT], FP32)
        for t in range(NT):
            nc.vector.reduce_sum(ent[:, t : t + 1], prod[:, t, :], axis=AX.X)
        aT = apool.tile([64, NT, 128], MMDT)          # aT[:, t, :] = a tile t transposed
        for g in range(2):
            tp = tpsum.tile([64, 4, 128], FP32, tag="tp")
            for j in range(4):
                nc.tensor.transpose(tp[:, j], a_nat[:, g * 4 + j, :], ident)
            nc.scalar.copy(aT[:, g * 4 : (g + 1) * 4, :], tp)

        # ---------------- chunk-pipelined main loop ----------------
        ev = 0
        for h in range(NCH):
            if h + 1 < NCH:
                b_tiles[(h + 1) % 2] = load_chunk(h + 1)
            b_nat = b_tiles[h % 2]
            # transpose raw b, fuse Ln into the PSUM->SBUF evacuation.
            # linear layout: column p of transpose j is b row h*1024+8p+j, so
            # the evac writes lbT columns {8p+j} (stride-8 view) and the GEMM
            # still reads m-contiguous 512-wide slices.
            lbT = lbtp.tile([64, CT * 128], MMDT, tag="lbT")
            lbT_v = lbT[:, :].rearrange("d (p j) -> d j p", j=CT)
            for g in range(CT // 4):
                tp = tpsum.tile([64, 4, 128], FP32, tag="tp")
                for j in range(4):
                    nc.tensor.transpose(tp[:, j], b_nat[:, g * 4 + j, :], ident)
                nc.scalar.activation(lbT_v[:, g * 4 : (g + 1) * 4, :], tp, AF.Ln)
            for t in range(NT):
                ps = mmps.tile([128, 2, 512], FP32, tag="ps")
                for j in range(2):
                    nc.tensor.matmul(
                        ps[:, j],
                        aT[:, t, :],
                        lbT[:, j * 512 : (j + 1) * 512],
                        start=True,
                        stop=True,
                    )
                out_sb = stage.tile([128, 2, 512], FP32, tag="out_sb")
                ent_t = ent[:, t : t + 1]
                # ACT picks up Ln-copies + half the DMA issue, so it gets 3
                # of 8 evacs; DVE the other 5.
                if ev % 8 in (0, 3, 6):
                    nc.scalar.activation(out_sb, ps, AF.Identity, bias=ent_t, scale=-1.0)
                else:
                    nc.vector.tensor_scalar(out_sb, ps, -1.0, ent_t, ALU.mult, ALU.add)
                # alternate the two HW DGE queues (SP / ACT) so per-
                # instruction queue re-arm bubbles hide behind the other
                # queue's transfers.
                dma_eng = nc.sync if ev % 2 == 0 else nc.scalar
                ev += 1
                dma_eng.dma_start(
                    out=out_d[
                        t * 128 : (t + 1) * 128, h * HCH : (h + 1) * HCH
                    ].rearrange("p (c m) -> p c m", m=512),
                    in_=out_sb,
                )
    # bacc lowering: splits multi-sem waits onto event-semaphore/nop
    # instructions (HW allows one sync wait per engine instruction).
    nc.compile()
    return nc


def _run(a, b, trace=False):
    from concourse.bass_utils import run_bass_kernel_spmd

    if MM_DTYPE not in _CACHE:
        _CACHE[MM_DTYPE] = _build(MM_DTYPE)
    nc = _CACHE[MM_DTYPE]
    a = np.ascontiguousarray(np.asarray(a, dtype=np.float32))
    b = np.ascontiguousarray(np.asarray(b, dtype=np.float32))
    in_maps = [
        {"a": a[i * NSHARD : (i + 1) * NSHARD], "b": b} for i in range(NCORES)
    ]
    res = run_bass_kernel_spmd(nc, in_maps, list(range(NCORES)), trace=trace)
    out = np.concatenate([r["out"] for r in res.results], axis=0)
    return out, res


def kernel(a, b):
    out, _ = _run(a, b, trace=False)
    return out


# revision 6
# speedup vs baseline: 1.0904x; 1.0904x over previous
"""Pairwise KL divergence kernel for Trainium2, SPMD across 8 NeuronCores.

out[n, m] = sum_d a[n,d]*(log a[n,d] - log b[m,d])
          = ent[n] - (a @ log(b)^T)[n, m],  ent = rowsum(a * log a)

Sharding: a (and output rows) split 8 ways; b replicated.
Per core: a_shard (1024, 64), b (8192, 64) -> out_shard (1024, 8192).

Structure (chunk-pipelined so the 32 MB output DMA starts ~6 us in and
never starves — it is the long pole at ~93 us of ~360 GB/s per core):
  - a prologue: load a natural (128p, 8t, 64d); la = Ln(a) [ACT];
    prod = a*la [DVE]; ent[128,8] per-tile reduce [DVE]; aT (f32r) via
    8 PE transposes + ACT copies.
  - for each of 8 b-chunks (1024 rows, double-buffered load):
      transpose the RAW chunk on PE, then one ACT Ln op per 4-tile group
      evacuates PSUM -> lbT (float32r) — Ln fused into the transpose
      evacuation, no separate lb tile or copies.
      for each of 8 n-tiles: 2 fp32r matmuls -> psum (128, 2, 512);
      evac fused with the entropy term (ACT Identity w/ bias, alternating
      with DVE tensor_scalar) -> 512 KB DMA to HBM (4 KB/partition lines).
"""

import numpy as np

N, M, D = 8192, 8192, 64
NCORES = 8
NSHARD = N // NCORES          # 1024 rows of a per core
NT = NSHARD // 128            # 8 n-tiles per core
HCH = 1024                    # b rows per chunk
NCH = M // HCH                # 8 chunks
CT = HCH // 128               # 8 b row-tiles per chunk

# matmul operand dtype: "fp32" (safe, 4 cyc/row) or "fp32r" (1 cyc/row)
MM_DTYPE = "fp32r"

_CACHE = {}


def _build(mm_dtype):
    from contextlib import ExitStack

    import concourse.bacc as bacc_mod
    import concourse.bass as bass
    import concourse.mybir as mybir
    import concourse.tile as tile
    from concourse.masks import make_identity

    FP32 = mybir.dt.float32
    AF = mybir.ActivationFunctionType
    ALU = mybir.AluOpType
    AX = mybir.AxisListType

    nc = bacc_mod.Bacc()
    a_d = nc.dram_tensor("a", [NSHARD, D], FP32, kind="ExternalInput")
    b_d = nc.dram_tensor("b", [M, D], FP32, kind="ExternalInput")
    out_d = nc.dram_tensor("out", [NSHARD, M], FP32, kind="ExternalOutput")

    # fp32r matmul operands must be *produced* as float32r (the BIR verifier
    # requires the producing instruction to round) — the ACT evacuations of
    # the PE transposes write float32r directly.
    MMDT = mybir.dt.float32r if mm_dtype == "fp32r" else FP32

    with tile.TileContext(nc) as tc, ExitStack() as ctx:
        consts = ctx.enter_context(tc.tile_pool(name="consts", bufs=1))
        apool = ctx.enter_context(tc.tile_pool(name="apool", bufs=1))
        bpool = ctx.enter_context(tc.tile_pool(name="bpool", bufs=2))
        lbtp = ctx.enter_context(tc.tile_pool(name="lbtp", bufs=2))
        tpsum = ctx.enter_context(tc.tile_pool(name="tpsum", bufs=2, space="PSUM"))
        mmps = ctx.enter_context(tc.tile_pool(name="mmps", bufs=3, space="PSUM"))
        stage = ctx.enter_context(tc.tile_pool(name="stage", bufs=6))

        ident = consts.tile([128, 128], FP32)
        make_identity(nc, ident)
        # Dummy transpose so PE observes the gpsimd (ident) sem here: the
        # matmul/LDW struct only carries ONE sync wait, so later transposes
        # must each need at most one sem (codegen: "Too many sync waits").
        warm = tpsum.tile([128, 128], FP32, tag="tp")
        nc.tensor.transpose(warm, ident, ident)

        # Linear chunk loads: partition p holds b rows h*1024 + 8p .. 8p+7 as
        # one contiguous 2 KB line (vs 256 B lines for the natural layout,
        # which eat ~10 us of DMA-engine time at the <512 B RMW penalty).
        b_r = b_d[:, :].rearrange("(c p j) d -> p c (j d)", c=NCH, p=128)

        def load_chunk(h):
            t = bpool.tile([128, CT, D], FP32, tag="b_nat")
            nc.sync.dma_start(
                out=t, in_=b_r[:, h, :].rearrange("p (j d) -> p j d", d=D)
            )
            return t

        # b chunk 0 in flight before the a prologue issues its DMA.
        b_tiles = [load_chunk(0), None]

        # ---------------- a prologue ----------------
        a_nat = apool.tile([128, NT, D], FP32)        # row t*128+p at [p, t, :]
        nc.sync.dma_start(out=a_nat, in_=a_d[:, :].rearrange("(t p) d -> p t d", p=128))
        la = apool.tile([128, NT, D], FP32)
        nc.scalar.activation(la, a_nat, AF.Ln)
        prod = apool.tile([128, NT, D], FP32)
        nc.vector.tensor_mul(prod, a_nat, la)
        ent = apool.tile([128, NT], FP32)
        for t in range(NT):
            nc.vector.reduce_sum(ent[:, t : t + 1], prod[:, t, :], axis=AX.X)
        aT = apool.tile([64, NT, 128], MMDT)          # aT[:, t, :] = a tile t transposed
        for g in range(2):
            tp = tpsum.tile([64, 4, 128], FP32, tag="tp")
            for j in range(4):
                nc.tensor.transpose(tp[:, j], a_nat[:, g * 4 + j, :], ident)
            nc.scalar.copy(aT[:, g * 4 : (g + 1) * 4, :], tp)

        # ---------------- chunk-pipelined main loop ----------------
        ev = 0
        for h in range(NCH):
            if h + 1 < NCH:
                b_tiles[(h + 1) % 2] = load_chunk(h + 1)
            b_nat = b_tiles[h % 2]
            # transpose raw b, fuse Ln into the PSUM->SBUF evacuation.
            # linear layout: column p of transpose j is b row h*1024+8p+j, so
            # the evac writes lbT columns {8p+j} (stride-8 view) and the GEMM
            # still reads m-contiguous 512-wide slices.
            lbT = lbtp.tile([64, CT * 128], MMDT, tag="lbT")
            lbT_v = lbT[:, :].rearrange("d (p j) -> d j p", j=CT)
            for g in range(CT // 4):
                tp = tpsum.tile([64, 4, 128], FP32, tag="tp")
                for j in range(4):
                    nc.tensor.transpose(tp[:, j], b_nat[:, g * 4 + j, :], ident)
                nc.scalar.activation(lbT_v[:, g * 4 : (g + 1) * 4, :], tp, AF.Ln)
            for t in range(NT):
                ps = mmps.tile([128, 2, 512], FP32, tag="ps")
                for j in range(2):
                    nc.tensor.matmul(
                        ps[:, j],
                        aT[:, t, :],
                        lbT[:, j * 512 : (j + 1) * 512],
                        start=True,
                        stop=True,
                    )
                out_sb = stage.tile([128, 2, 512], FP32, tag="out_sb")
                ent_t = ent[:, t : t + 1]
                # ACT picks up Ln-copies + half the DMA issue, so it gets 3
                # of 8 evacs; DVE the other 5.
                if ev % 8 in (0, 3, 6):
                    nc.scalar.activation(out_sb, ps, AF.Identity, bias=ent_t, scale=-1.0)
                else:
                    nc.vector.tensor_scalar(out_sb, ps, -1.0, ent_t, ALU.mult, ALU.add)
                # alternate the two HW DGE queues (SP / ACT) so per-
                # instruction queue re-arm bubbles hide behind the other
                # queue's transfers.
                dma_eng = nc.sync if ev % 2 == 0 else nc.scalar
                ev += 1
                dma_eng.dma_start(
                    out=out_d[
                        t * 128 : (t + 1) * 128, h * HCH : (h + 1) * HCH
                    ].rearrange("p (c m) -> p c m", m=512),
                    in_=out_sb,
                )
    # bacc lowering: splits multi-sem waits onto event-semaphore/nop
    # instructions (HW allows one sync wait per engine instruction).
    nc.compile()
    return nc


def _run(a, b, trace=False):
    from concourse.bass_utils import run_bass_kernel_spmd

    if MM_DTYPE not in _CACHE:
        _CACHE[MM_DTYPE] = _build(MM_DTYPE)
    nc = _CACHE[MM_DTYPE]
    a = np.ascontiguousarray(np.asarray(a, dtype=np.float32))
    b = np.ascontiguousarray(np.asarray(b, dtype=np.float32))
    in_maps = [
        {"a": a[i * NSHARD : (i + 1) * NSHARD], "b": b} for i in range(NCORES)
    ]
    res = run_bass_kernel_spmd(nc, in_maps, list(range(NCORES)), trace=trace)
    out = np.concatenate([r["out"] for r in res.results], axis=0)
    return out, res


def kernel(a, b):
    out, _ = _run(a, b, trace=False)
    return out


# revision 7
# speedup vs baseline: 1.1058x; 1.0141x over previous
"""Pairwise KL divergence kernel for Trainium2, SPMD across 8 NeuronCores.

out[n, m] = sum_d a[n,d]*(log a[n,d] - log b[m,d])
          = ent[n] - (a @ log(b)^T)[n, m],  ent = rowsum(a * log a)

Sharding: a (and output rows) split 8 ways; b replicated.
Per core: a_shard (1024, 64), b (8192, 64) -> out_shard (1024, 8192).

Structure (chunk-pipelined so the 32 MB output DMA starts ~6 us in and
never starves — it is the long pole at ~93 us of ~360 GB/s per core):
  - a prologue: load a natural (128p, 8t, 64d); la = Ln(a) [ACT];
    prod = a*la [DVE]; ent[128,8] per-tile reduce [DVE]; aT (f32r) via
    8 PE transposes + ACT copies.
  - for each of 8 b-chunks (1024 rows, double-buffered load):
      transpose the RAW chunk on PE, then one ACT Ln op per 4-tile group
      evacuates PSUM -> lbT (float32r) — Ln fused into the transpose
      evacuation, no separate lb tile or copies.
      for each of 8 n-tiles: 2 fp32r matmuls -> psum (128, 2, 512);
      evac fused with the entropy term (ACT Identity w/ bias, alternating
      with DVE tensor_scalar) -> 512 KB DMA to HBM (4 KB/partition lines).
"""

import numpy as np

N, M, D = 8192, 8192, 64
NCORES = 8
NSHARD = N // NCORES          # 1024 rows of a per core
NT = NSHARD // 128            # 8 n-tiles per core
HCH = 1024                    # b rows per chunk
NCH = M // HCH                # 8 chunks
CT = HCH // 128               # 8 b row-tiles per chunk

# matmul operand dtype: "fp32" (safe, 4 cyc/row) or "fp32r" (1 cyc/row)
MM_DTYPE = "fp32r"

_CACHE = {}


def _build(mm_dtype):
    from contextlib import ExitStack

    import concourse.bacc as bacc_mod
    import concourse.bass as bass
    import concourse.mybir as mybir
    import concourse.tile as tile
    from concourse.masks import make_identity

    FP32 = mybir.dt.float32
    AF = mybir.ActivationFunctionType
    ALU = mybir.AluOpType
    AX = mybir.AxisListType

    nc = bacc_mod.Bacc()
    a_d = nc.dram_tensor("a", [NSHARD, D], FP32, kind="ExternalInput")
    b_d = nc.dram_tensor("b", [M, D], FP32, kind="ExternalInput")
    out_d = nc.dram_tensor("out", [NSHARD, M], FP32, kind="ExternalOutput")

    # fp32r matmul operands must be *produced* as float32r (the BIR verifier
    # requires the producing instruction to round) — the ACT evacuations of
    # the PE transposes write float32r directly.
    MMDT = mybir.dt.float32r if mm_dtype == "fp32r" else FP32

    with tile.TileContext(nc) as tc, ExitStack() as ctx:
        consts = ctx.enter_context(tc.tile_pool(name="consts", bufs=1))
        apool = ctx.enter_context(tc.tile_pool(name="apool", bufs=1))
        bpool = ctx.enter_context(tc.tile_pool(name="bpool", bufs=2))
        lbtp = ctx.enter_context(tc.tile_pool(name="lbtp", bufs=2))
        tpsum = ctx.enter_context(tc.tile_pool(name="tpsum", bufs=2, space="PSUM"))
        mmps = ctx.enter_context(tc.tile_pool(name="mmps", bufs=3, space="PSUM"))
        stage = ctx.enter_context(tc.tile_pool(name="stage", bufs=6))

        ident = consts.tile([128, 128], FP32)
        make_identity(nc, ident)
        # Dummy transpose so PE observes the gpsimd (ident) sem here: the
        # matmul/LDW struct only carries ONE sync wait, so later transposes
        # must each need at most one sem (codegen: "Too many sync waits").
        warm = tpsum.tile([128, 128], FP32, tag="tp")
        nc.tensor.transpose(warm, ident, ident)

        # Linear chunk loads: partition p holds b rows h*1024 + 8p .. 8p+7 as
        # one contiguous 2 KB line (vs 256 B lines for the natural layout,
        # which eat ~10 us of DMA-engine time at the <512 B RMW penalty).
        b_r = b_d[:, :].rearrange("(c p j) d -> p c (j d)", c=NCH, p=128)

        def load_chunk(h):
            t = bpool.tile([128, CT, D], FP32, tag="b_nat")
            nc.sync.dma_start(
                out=t, in_=b_r[:, h, :].rearrange("p (j d) -> p j d", d=D)
            )
            return t

        # b chunk 0 in flight before the a prologue issues its DMA.
        b_tiles = [load_chunk(0), None]

        # ---------------- a prologue ----------------
        a_nat = apool.tile([128, NT, D], FP32)        # row t*128+p at [p, t, :]
        nc.sync.dma_start(out=a_nat, in_=a_d[:, :].rearrange("(t p) d -> p t d", p=128))
        la = apool.tile([128, NT, D], FP32)
        nc.scalar.activation(la, a_nat, AF.Ln)
        prod = apool.tile([128, NT, D], FP32)
        nc.vector.tensor_mul(prod, a_nat, la)
        ent = apool.tile([128, NT], FP32)
        for t in range(NT):
            nc.vector.reduce_sum(ent[:, t : t + 1], prod[:, t, :], axis=AX.X)
        aT = apool.tile([64, NT, 128], MMDT)          # aT[:, t, :] = a tile t transposed
        for g in range(2):
            tp = tpsum.tile([64, 4, 128], FP32, tag="tp")
            for j in range(4):
                nc.tensor.transpose(tp[:, j], a_nat[:, g * 4 + j, :], ident)
            # DVE evacuation keeps ACT free for Ln work in the head
            nc.vector.tensor_copy(aT[:, g * 4 : (g + 1) * 4, :], tp)

        # ---------------- chunk-pipelined main loop ----------------
        ev = 0
        for h in range(NCH):
            if h + 1 < NCH:
                b_tiles[(h + 1) % 2] = load_chunk(h + 1)
            b_nat = b_tiles[h % 2]
            # transpose raw b, fuse Ln into the PSUM->SBUF evacuation.
            # linear layout: column p of transpose j is b row h*1024+8p+j, so
            # the evac writes lbT columns {8p+j} (stride-8 view) and the GEMM
            # still reads m-contiguous 512-wide slices.
            lbT = lbtp.tile([64, CT * 128], MMDT, tag="lbT")
            lbT_v = lbT[:, :].rearrange("d (p j) -> d j p", j=CT)
            for g in range(CT // 4):
                tp = tpsum.tile([64, 4, 128], FP32, tag="tp")
                for j in range(4):
                    nc.tensor.transpose(tp[:, j], b_nat[:, g * 4 + j, :], ident)
                nc.scalar.activation(lbT_v[:, g * 4 : (g + 1) * 4, :], tp, AF.Ln)
            for t in range(NT):
                ps = mmps.tile([128, 2, 512], FP32, tag="ps")
                for j in range(2):
                    nc.tensor.matmul(
                        ps[:, j],
                        aT[:, t, :],
                        lbT[:, j * 512 : (j + 1) * 512],
                        start=True,
                        stop=True,
                    )
                out_sb = stage.tile([128, 2, 512], FP32, tag="out_sb")
                ent_t = ent[:, t : t + 1]
                # ACT picks up Ln-copies + half the DMA issue, so it gets 3
                # of 8 evacs; DVE the other 5.
                if ev % 8 in (0, 3, 6):
                    nc.scalar.activation(out_sb, ps, AF.Identity, bias=ent_t, scale=-1.0)
                else:
                    nc.vector.tensor_scalar(out_sb, ps, -1.0, ent_t, ALU.mult, ALU.add)
                # alternate the two HW DGE queues (SP / ACT) so per-
                # instruction queue re-arm bubbles hide behind the other
                # queue's transfers.
                dma_eng = nc.sync if ev % 2 == 0 else nc.scalar
                ev += 1
                dma_eng.dma_start(
                    out=out_d[
                        t * 128 : (t + 1) * 128, h * HCH : (h + 1) * HCH
                    ].rearrange("p (c m) -> p c m", m=512),
                    in_=out_sb,
                )
    # bacc lowering: splits multi-sem waits onto event-semaphore/nop
    # instructions (HW allows one sync wait per engine instruction).
    nc.compile()
    return nc


def _run(a, b, trace=False):
    from concourse.bass_utils import run_bass_kernel_spmd

    if MM_DTYPE not in _CACHE:
        _CACHE[MM_DTYPE] = _build(MM_DTYPE)
    nc = _CACHE[MM_DTYPE]
    a = np.ascontiguousarray(np.asarray(a, dtype=np.float32))
    b = np.ascontiguousarray(np.asarray(b, dtype=np.float32))
    in_maps = [
        {"a": a[i * NSHARD : (i + 1) * NSHARD], "b": b} for i in range(NCORES)
    ]
    res = run_bass_kernel_spmd(nc, in_maps, list(range(NCORES)), trace=trace)
    out = np.concatenate([r["out"] for r in res.results], axis=0)
    return out, res


def kernel(a, b):
    out, _ = _run(a, b, trace=False)
    return out
